# revision 3
# baseline (speedup 1.0000x reference)
"""MiniMax MoE gate (sigmoid + bias, top-8, normalized weights) on 8 TRN2 cores.

Full inputs in, full outputs out. Data-parallel over the token dim: each of
the 8 NeuronCores handles 1024 of the 8192 tokens; the [64, 4096] gate weight
and [64] bias are replicated.

Per-core plan (tokens on SBUF partitions throughout):
  - gate_weight is loaded [64, 4096] and PE-transposed once into wT [128, 32, 64]
    (d-chunk on partitions).
  - x is streamed in [128, 512] tiles, PE-transposed (fp32 transpose mode) via
    PSUM, and fed as the matmul stationary operand; wT chunks are the moving
    operand, accumulating logits [128 tok, 64 expert] in PSUM over 32 k-chunks.
  - sigmoid is computed precisely as e^l / (1 + e^l) (ACT exp is ~2 ULP vs the
    sigmoid LUT's 40-ULP budget; top-8 selection on `sigmoid + bias` is
    rounding-sensitive, so the extra DVE divide is worth it).
  - hardware top-8: nc.vector.max (8 largest, descending) + max_index (their
    indices, ties resolved to ascending index) — exactly jax.lax.top_k order.
  - per-slot raw scores are recovered with one fused scalar_tensor_tensor per
    slot: accum_k = sum_e (biased == msel_k) * score, then weights = ssel/sum.
"""

import numpy as np

import concourse.bacc as bacc
import concourse.mybir as mybir
from concourse.bass_utils import run_bass_kernel_spmd
from concourse.masks import make_identity
from concourse.tile import TileContext

T, D, E, K = 8192, 4096, 64, 8
NCORES = 8
P = 128
XCOLS = 512  # x DMA tile width (2 KB per partition per descriptor)
F32 = mybir.dt.float32


def build_nc(ts):
    """Build the per-core Bass program for a shard of `ts` tokens."""
    nt = ts // P        # token tiles
    dc = D // P         # 32 contraction chunks
    nx = D // XCOLS     # x loads per token tile
    qpl = XCOLS // P    # transposes per x load

    nc = bacc.Bacc("TRN2", target_bir_lowering=False)
    x = nc.dram_tensor("x", [ts, D], F32, kind="ExternalInput")
    w = nc.dram_tensor("gate_weight", [E, D], F32, kind="ExternalInput")
    b = nc.dram_tensor("bias", [1, E], F32, kind="ExternalInput")
    oi = nc.dram_tensor("out_idx", [ts, K], mybir.dt.int32, kind="ExternalOutput")
    ow = nc.dram_tensor("out_w", [ts, K], F32, kind="ExternalOutput")

    with TileContext(nc) as tc:
        with (
            tc.tile_pool(name="const", bufs=1) as cpool,
            tc.tile_pool(name="wts", bufs=1) as wpool,
            tc.tile_pool(name="xin", bufs=4) as xpool,
            tc.tile_pool(name="xt", bufs=3) as xtpool,
            tc.tile_pool(name="epi", bufs=2) as epool,
            tc.tile_pool(name="pstage", bufs=2, space="PSUM") as pstage,
            tc.tile_pool(name="plogit", bufs=2, space="PSUM") as plogit,
        ):
            ident = cpool.tile([P, P], F32)
            make_identity(nc, ident)

            bias_row = cpool.tile([1, E], F32)
            nc.sync.dma_start(out=bias_row, in_=b[:, :])
            bias_bc = cpool.tile([P, E], F32)
            nc.gpsimd.partition_broadcast(bias_bc, bias_row)

            # Gate weight: [64, 4096] rows on 64 partitions, then transpose
            # 128-wide chunks through PSUM into wT [128, 32, 64].
            w_sb = wpool.tile([E, D], F32)
            nc.sync.dma_start(out=w_sb, in_=w[:, :])
            wT = wpool.tile([P, dc, E], F32)
            for g in range(dc // 8):
                ps = pstage.tile([P, XCOLS], F32, tag="stage")
                for j in range(8):
                    c = g * 8 + j
                    nc.tensor.matmul(
                        ps[:, j * E:(j + 1) * E],
                        w_sb[:, c * P:(c + 1) * P],
                        ident[:E, :E],
                        is_transpose=True,
                        start=(j == 0),
                        stop=(j == 7),
                    )
                nc.scalar.copy(out=wT[:, g * 8:(g + 1) * 8, :], in_=ps)

            for bt in range(nt):
                lg = plogit.tile([P, E], F32, tag="logits")
                for xl in range(nx):
                    xs = xpool.tile([P, XCOLS], F32, tag="x")
                    nc.sync.dma_start(
                        out=xs,
                        in_=x[bt * P:(bt + 1) * P, xl * XCOLS:(xl + 1) * XCOLS],
                    )
                    ps = pstage.tile([P, XCOLS], F32, tag="stage")
                    for q in range(qpl):
                        nc.tensor.matmul(
                            ps[:, q * P:(q + 1) * P],
                            xs[:, q * P:(q + 1) * P],
                            ident,
                            is_transpose=True,
                            start=(q == 0),
                            stop=(q == qpl - 1),
                        )
                    xts = xtpool.tile([P, XCOLS], F32, tag="xt")
                    nc.scalar.copy(out=xts, in_=ps)
                    for q in range(qpl):
                        c = xl * qpl + q
                        nc.tensor.matmul(
                            lg,
                            xts[:, q * P:(q + 1) * P],
                            wT[:, c, :],
                            start=(c == 0),
                            stop=(c == dc - 1),
                        )

                # epilogue: scores, biased, top-8, per-slot extraction, normalize
                # sigmoid = 1 / (1 + e^-l): ACT exp (~2 ULP) + DVE reciprocal
                ex = epool.tile([P, E], F32, tag="ex")
                nc.scalar.activation(
                    out=ex, in_=lg, func=mybir.ActivationFunctionType.Exp,
                    scale=-1.0,
                )
                den = epool.tile([P, E], F32, tag="den")
                nc.vector.tensor_scalar_add(den, ex, 1.0)
                sc = epool.tile([P, E], F32, tag="sc")
                nc.vector.reciprocal(out=sc, in_=den)
                bi = epool.tile([P, E], F32, tag="bi")
                nc.vector.tensor_tensor(
                    out=bi, in0=sc, in1=bias_bc, op=mybir.AluOpType.add
                )
                msel = epool.tile([P, K], F32, tag="msel")
                nc.vector.max(out=msel, in_=bi)
                idxu = epool.tile([P, K], mybir.dt.uint32, tag="idxu")
                nc.vector.max_index(out=idxu, in_max=msel, in_values=bi)
                ssel = epool.tile([P, K], F32, tag="ssel")
                for k in range(K):
                    scr = epool.tile([P, E], F32, tag="scr")
                    nc.vector.scalar_tensor_tensor(
                        out=scr,
                        in0=bi,
                        scalar=msel[:, k:k + 1],
                        in1=sc,
                        op0=mybir.AluOpType.is_equal,
                        op1=mybir.AluOpType.mult,
                        accum_out=ssel[:, k:k + 1],
                    )
                ssum = epool.tile([P, 1], F32, tag="ssum")
                nc.vector.tensor_reduce(
                    out=ssum,
                    in_=ssel,
                    axis=mybir.AxisListType.X,
                    op=mybir.AluOpType.add,
                )
                rsum = epool.tile([P, 1], F32, tag="rsum")
                nc.vector.reciprocal(out=rsum, in_=ssum)
                wo = epool.tile([P, K], F32, tag="wo")
                nc.vector.tensor_scalar_mul(wo, ssel, rsum[:])
                nc.sync.dma_start(
                    out=oi[bt * P:(bt + 1) * P, :],
                    in_=idxu[:].bitcast(mybir.dt.int32),
                )
                nc.sync.dma_start(out=ow[bt * P:(bt + 1) * P, :], in_=wo)

    nc.compile()
    return nc


_NC_CACHE = {}


def _get_nc(ts):
    if ts not in _NC_CACHE:
        _NC_CACHE[ts] = build_nc(ts)
    return _NC_CACHE[ts]


def kernel(x, gate_weight, bias):
    x = np.ascontiguousarray(np.asarray(x, dtype=np.float32))
    gw = np.ascontiguousarray(np.asarray(gate_weight, dtype=np.float32))
    bb = np.ascontiguousarray(np.asarray(bias, dtype=np.float32)).reshape(1, E)

    ts = T // NCORES
    nc = _get_nc(ts)
    shards = np.split(x, NCORES, axis=0)
    in_maps = [{"x": s, "gate_weight": gw, "bias": bb} for s in shards]
    res = run_bass_kernel_spmd(nc, in_maps, core_ids=list(range(NCORES)))
    idx = np.concatenate([r["out_idx"] for r in res.results], axis=0)
    wts = np.concatenate([r["out_w"] for r in res.results], axis=0)
    return idx, wts
